# revision 37
# baseline (speedup 1.0000x reference)
"""AttentionBlock (1x1-conv QKV + 4-head softmax attention + 1x1-conv proj)
on 8 Trainium2 NeuronCores.

Sharding: data-parallel over (batch b, query-half h) -> 8 shards. Each core
gets x rotated so its 2048 query columns are always columns 0:2048 (key order
is a permutation, which softmax-attention is invariant to), computes
qkv projections, 4-head attention for its half of the queries, and the output
projection for its [256, 2048] output slice. No collectives.

v2 structure (cost-model aware: matmul cost = streamed rhs columns):
  - scores S^T = K^T Q in f32r (1/16 pre-folded into w_q on the host so the
    DVE exp polynomial stays in range), 256-query tiles, keys-major PSUM.
    q/k live in per-head partition-0 tiles: mixed-partition-offset matmul
    operands crash the walrus/HW path.
  - exp split by column between Act (native Exp, scale=16) and a two-instr
    DVE pipeline (EXPQ2A: minimax-quartic^2 of exp(16t)*24, EXPQ2B: ^8),
    24^16 cancels in softmax since rowsums come from the same values. The
    per-nt column split keeps every softmax row on one implementation.
  - attn@V in O-form: out[query, dh] with rhs=[V_h | ones] so rowsums ride
    along as a 65th column; 65-col bf16 matmuls with 128-query-partition
    output (2x fewer streamed columns than the channel-major form). One
    PSUM accumulation group per 2KB bank (lazy zero-region semantics).
  - normalization per 128-query chunk on DVE (reciprocal + stride-0-broadcast
    tensor_tensor), then PE transposes O back to channel-major (identity
    rhs) for the output projection, pipelined per 512-query group.
  - f32r DRAM params + f32r SBUF tiles everywhere (no conversion copies);
    PSUM triple-buffered scores so the exp WAR chain stays off the critical
    path; EXPQ2B deprioritized so the next tile's EXPQ2A fills its ack gap.
"""
import os
import sys

sys.path.insert(0, '/opt/trn_rl_repo')

import numpy as np
from contextlib import ExitStack

from concourse import bass, bacc, mybir
import concourse.tile as tile
from concourse import dve_ops
from concourse.dve_ops import DveOp, OPS, CUSTOM_DVE_SPECS, _SUB_OPCODE_FOR_NAME
from concourse.dve_spec import Spec, Src0, C0, C1, C2, C3, lower, sq, _spill_c3_to_src1
from concourse.dve_uop import DveOpSpec
from concourse.bass_utils import run_bass_kernel_spmd

F32 = mybir.dt.float32
F32R = mybir.dt.float32r
BF16 = mybir.dt.bfloat16
ActFn = mybir.ActivationFunctionType

B, C, H, W = 4, 256, 64, 64
HEADS, DH = 4, 64
N = H * W            # 4096 keys
NQ = N // 2          # 2048 queries per core
NT = 256             # phase-2 query tile
N_NT = NQ // NT      # 8
N_MC = N // 128      # 32 key chunks
VSTR = HEADS * (DH + 1)  # 260: per-mc vT stride ([V_h | ones] x 4 heads)

# exp(16t) * 24^16 ~ [(t^2 + c0 t + c1)(t^2 + c2 t + c3)]^16 for t in
# [-0.625, 0.625] (score x = 16t in [-10, 10]); max rel err ~9e-4. The
# 24^16 factor cancels in softmax normalization. Split into two DVE
# instructions: EXPQ2A computes P^2 (quartic + one square, 8 ALU ops),
# EXPQ2B cubes the squaring three more times ((P^2)^8 = P^16).
EQ = (0.5504330780327099, 6.148042182109957,
      3.5525352677618507, 3.903596315668177)

# Act exp column count (0..1024) per nt slot; the remaining columns of every
# score tile go to the DVE quartic pipeline. Uniform split keeps both engines
# busy on every tile (whole-tile alternation serializes the engines in time).
EXP_ACOLS = [int(v) for v in os.environ.get(
    "EXP_ACOLS", "765,765,765,765,765,765,765,765").split(",")]
assert len(EXP_ACOLS) == 8


def _ref_expq2a(in0, in1, c0, c1, c2):
    x = in0.astype(np.float32)
    c3 = in1.astype(np.float32) if isinstance(in1, np.ndarray) else np.float32(in1)
    p = (((x + np.float32(c0)) * x + np.float32(c1))
         * ((x + np.float32(c2)) * x + c3)).astype(np.float32)
    return (p * p).astype(np.float32)


def _ref_expq2b(in0, in1, c0, c1, c2):
    p = in0.astype(np.float32)
    for _ in range(3):
        p = (p * p).astype(np.float32)
    return p


def _register(name, spec, rd1_en):
    row = dve_ops._CUSTOM_DVE_ROW_BASE + len(OPS)
    assert row < 0x20
    _SUB_OPCODE_FOR_NAME[name] = row
    shas = {}
    for ver in ("v3", "v4"):
        uops = lower(spec, ver=ver)
        shas[ver] = DveOpSpec(name=name, opcode=row, uops=uops, rd1_en=rd1_en).sha(ver)
    op = DveOp(name, spec, subdim=False, uops_sha=shas)
    OPS.append(op)
    CUSTOM_DVE_SPECS[name] = spec
    return op


def register_expq_op():
    if "EXPQ2A_ANT" in _SUB_OPCODE_FOR_NAME:
        a = next(op for op in OPS if op.name == "EXPQ2A_ANT")
        b = next(op for op in OPS if op.name == "EXPQ2B_ANT")
        return a, b
    x = Src0
    body_a = _spill_c3_to_src1(
        sq(((x + C0) * x + C1) * ((x + C2) * x + C3)))
    op_a = _register("EXPQ2A_ANT", Spec(body=body_a, reference=_ref_expq2a), True)
    body_b = sq(sq(sq(x)))
    op_b = _register("EXPQ2B_ANT", Spec(body=body_b, reference=_ref_expq2b), False)
    return op_a, op_b


def _ap3(base_ap, dims):
    """Manual AP with the partition dim of base_ap plus custom free dims."""
    return bass.AP(tensor=base_ap.tensor, offset=base_ap.offset,
                   ap=[list(base_ap.ap[0])] + [list(d) for d in dims])


def build_program(expq_op):
    nc = bacc.Bacc(target_bir_lowering=False)

    x_d = nc.declare_dram_parameter("x", [C, N], F32R, isOutput=False)
    wq_d = nc.declare_dram_parameter("wq", [C, C], F32R, isOutput=False)
    wk_d = nc.declare_dram_parameter("wk", [C, C], F32R, isOutput=False)
    wv_d = nc.declare_dram_parameter("wv", [C, C], F32R, isOutput=False)
    wp_d = nc.declare_dram_parameter("wp", [C, C], F32R, isOutput=False)
    bias_d = nc.declare_dram_parameter("bias", [128, 2], F32, isOutput=False)
    id_d = nc.declare_dram_parameter("ident", [128, 128], F32R, isOutput=False)
    y_d = nc.declare_dram_parameter("y", [C, NQ], F32, isOutput=True)

    with tile.TileContext(nc) as tc, ExitStack() as ctx:
        sb = ctx.enter_context(tc.tile_pool(name="sb", bufs=1))
        pex = ctx.enter_context(tc.tile_pool(name="pex", bufs=3))
        pout = ctx.enter_context(tc.tile_pool(name="pout", bufs=2))
        ps = ctx.enter_context(tc.tile_pool(name="ps", bufs=1, space="PSUM"))

        # ---------------- loads (weights first so QKV can start early) -----
        XC = 512  # x DMA chunk width so phase 1 can start early
        w_sb = {}
        w_drams = {"wq": wq_d, "wk": wk_d, "wv": wv_d, "wp": wp_d}
        for name in w_drams:
            w_sb[name] = [sb.tile([128, C], F32R, tag=f"{name}{kc}", name=f"{name}f{kc}")
                          for kc in range(2)]
        x_f = [sb.tile([128, N], F32R, tag=f"xf{i}", name=f"xf{i}") for i in range(2)]

        def w_dma(name):
            for kc in range(2):
                nc.sync.dma_start(out=w_sb[name][kc],
                                  in_=w_drams[name][kc * 128:(kc + 1) * 128, :])

        def x_dma(ch):
            for kc in range(2):
                nc.sync.dma_start(out=x_f[kc][:, ch * XC:(ch + 1) * XC],
                                  in_=x_d[kc * 128:(kc + 1) * 128, ch * XC:(ch + 1) * XC])

        w_dma("wq")
        x_dma(0)
        w_dma("wk")
        x_dma(1)
        w_dma("wv")
        w_dma("wp")
        for ch in range(2, N // XC):
            x_dma(ch)
        bias_sb = sb.tile([128, 2], F32, tag="bias")
        nc.sync.dma_start(out=bias_sb, in_=bias_d[:, :])
        id_sb = sb.tile([128, 128], F32R, tag="id")
        nc.sync.dma_start(out=id_sb, in_=id_d[:, :])

        c3_t = sb.tile([128, 1], F32, tag="c3")
        nc.vector.memset(c3_t, float(EQ[3]))

        def xr(kc, sl):
            return x_f[kc][:, sl]

        def wr(name, kc, oc):
            return w_sb[name][kc][:, oc * 128:(oc + 1) * 128]

        # ---------------- phase 1: qkv projections ----------------
        # per-head tiles, always at partition offset 0 (mixed-partition-offset
        # matmul operands crash the walrus/HW path)
        q_sb = [sb.tile([64, NQ], F32R, tag=f"q{h}", name=f"q_sb{h}") for h in range(4)]
        k_sb = [sb.tile([64, N], F32R, tag=f"k{h}", name=f"k_sb{h}") for h in range(4)]
        vT_sb = sb.tile([128, N_MC * VSTR], BF16, tag="vT")

        # ones columns of vT (col 64 + 65*h + 260*mc), written once on Pool
        ones_ap = _ap3(vT_sb[:, DH:DH + 1], [[VSTR, N_MC], [DH + 1, HEADS]])
        nc.gpsimd.memset(ones_ap, 1.0)

        evac_i = [0]

        def evac_copy(out_ap, in_ap):
            # alternate PSUM evacuations between Act and DVE
            eng = nc.scalar.copy if evac_i[0] % 2 == 0 else nc.vector.tensor_copy
            evac_i[0] += 1
            return eng(out_ap, in_ap)

        for oc in range(2):
            for t4 in range(4):
                pq = ps.tile([128, 512], F32, tag="st", bufs=3, name=f"pq{oc}_{t4}")
                sl = slice(t4 * 512, (t4 + 1) * 512)
                nc.tensor.matmul(out=pq[:, :], lhsT=wr("wq", 0, oc), rhs=xr(0, sl),
                                 start=True, stop=False)
                nc.tensor.matmul(out=pq[:, :], lhsT=wr("wq", 1, oc), rhs=xr(1, sl),
                                 start=False, stop=True)
                evac_copy(q_sb[2 * oc][:, sl], pq[0:64, :])
                evac_copy(q_sb[2 * oc + 1][:, sl], pq[64:128, :])
        for oc in range(2):
            for t8 in range(8):
                pk = ps.tile([128, 512], F32, tag="st", bufs=3, name=f"pk{oc}_{t8}")
                sl = slice(t8 * 512, (t8 + 1) * 512)
                nc.tensor.matmul(out=pk[:, :], lhsT=wr("wk", 0, oc), rhs=xr(0, sl),
                                 start=True, stop=False)
                nc.tensor.matmul(out=pk[:, :], lhsT=wr("wk", 1, oc), rhs=xr(1, sl),
                                 start=False, stop=True)
                evac_copy(k_sb[2 * oc][:, sl], pk[0:64, :])
                evac_copy(k_sb[2 * oc + 1][:, sl], pk[64:128, :])
        for mc in range(N_MC):
            pv = ps.tile([128, 256], F32, tag="st", bufs=3, name=f"pv{mc}")
            msl = slice(mc * 128, (mc + 1) * 128)
            nc.tensor.matmul(out=pv[:, :], lhsT=xr(0, msl), rhs=w_sb["wv"][0][:, :],
                             start=True, stop=False)
            nc.tensor.matmul(out=pv[:, :], lhsT=xr(1, msl), rhs=w_sb["wv"][1][:, :],
                             start=False, stop=True)
            # strided copy into the [V_h | ones] layout: col 65*h + d
            vout = _ap3(vT_sb[:, mc * VSTR:mc * VSTR + 1], [[DH + 1, HEADS], [1, DH]])
            vin = _ap3(pv[:, 0:1], [[DH, HEADS], [1, DH]])
            evac_copy(vout, vin)

        # ---------------- phase 2: attention ----------------
        o_n = sb.tile([128, 16 * 256], F32R, tag="on")   # normalized O, [q, c]
        out_sp = [sb.tile([128, NQ], F32R, tag=f"osp{oc}", name=f"osp{oc}") for oc in range(2)]

        op_a, op_b = expq_op
        for nt in range(N_NT):               # 256-query tiles
            qsl = slice(nt * NT, (nt + 1) * NT)
            O_ps = [ps.tile([128, 512], F32, tag="o", bufs=2, name=f"O{nt}_{qs}")
                    for qs in range(2)]
            for mc in range(N_MC):
                msl = slice(mc * 128, (mc + 1) * 128)
                # all 4 heads' scores for this (nt, mc) in one 2-bank tile;
                # triple-buffered so the exp WAR chain stays off the
                # critical path.
                pst = ps.tile([128, 1024], F32, tag="st", bufs=3,
                              name=f"pst{nt}_{mc}")
                for h in range(4):
                    # per-head operands at partition offset 0; two heads per
                    # 2KB PSUM bank: first starts the group (lazy-zeroing the
                    # bank), second stops it.
                    nc.tensor.matmul(out=pst[:, h * 256:(h + 1) * 256],
                                     lhsT=k_sb[h][:, msl],
                                     rhs=q_sb[h][:, qsl],
                                     start=(h % 2 == 0), stop=(h % 2 == 1))
                et = pex.tile([128, 1024], BF16, tag="et", name=f"et{nt}_{mc}")
                acols = EXP_ACOLS[nt]
                if acols > 0:
                    nc.scalar.activation(et[:, 0:acols], pst[:, 0:acols],
                                         ActFn.Exp, scale=16.0)
                if acols < 1024:
                    y1 = pex.tile([128, 1024], F32, tag="y1", name=f"y1{nt}_{mc}")
                    nc.vector._custom_dve(op_a, out=y1[:, acols:1024],
                                          in0=pst[:, acols:1024],
                                          in1=c3_t[:, :], s0=float(EQ[0]),
                                          s1=float(EQ[1]), imm2=float(EQ[2]))
                    # deprioritize the second stage so the scheduler slots the
                    # next tile's EXPQ2A into the A->B ack gap instead of
                    # idling the DVE on the y1 write-ack.
                    with tc.high_priority(-24):
                        nc.vector._custom_dve(op_b, out=et[:, acols:1024],
                                              in0=y1[:, acols:1024])
                first, last = mc == 0, mc == N_MC - 1
                for h in range(4):
                    for qs in range(2):
                        # one accumulation group per O bank: start only on the
                        # very first write (the zero-region covers all 4 heads'
                        # columns), stop only on the very last.
                        nc.tensor.matmul(
                            out=O_ps[qs][:, h * 128:h * 128 + DH + 1],
                            lhsT=et[:, h * 256 + qs * 128:h * 256 + qs * 128 + 128],
                            rhs=vT_sb[:, mc * VSTR + h * (DH + 1):mc * VSTR + (h + 1) * (DH + 1)],
                            start=(first and h == 0), stop=(last and h == 3))
            for qs in range(2):
                rcp = sb.tile([128, 4], F32, tag="rcp", bufs=2, name=f"rcp{nt}_{qs}")
                rs_ap = _ap3(O_ps[qs][:, DH:DH + 1], [[128, 4], [1, 1]])
                nc.vector.reciprocal_approx_fast(out=rcp[:, :], in_=rs_ap)
                qc = nt * 2 + qs
                o_out = _ap3(o_n[:, qc * 256:qc * 256 + 1], [[64, 4], [1, 64]])
                o_in = _ap3(O_ps[qs][:, 0:1], [[128, 4], [1, 64]])
                r_in = _ap3(rcp[:, 0:1], [[1, 4], [0, 64]])
                nc.vector.tensor_tensor(out=o_out, in0=o_in, in1=r_in,
                                        op=mybir.AluOpType.mult)
            if nt % 2 == 1:
                # transpose the last 4 qchunks back to channel-major and
                # project, pipelined with the next nt's attention. Deprioritized
                # so the next nt's S^T matmuls win the PE when both are ready.
                ctx.enter_context(tc.high_priority(-64))
                g = nt // 2
                sl = slice(g * 512, (g + 1) * 512)
                for cc in range(2):
                    psT = ps.tile([128, 512], F32R, tag="o", bufs=2,
                                  name=f"psT{g}_{cc}")
                    for j in range(4):
                        qc = g * 4 + j
                        nc.tensor.matmul(
                            out=psT[:, j * 128:(j + 1) * 128],
                            lhsT=o_n[:, qc * 256 + cc * 128:qc * 256 + cc * 128 + 128],
                            rhs=id_sb[:, :],
                            is_transpose=True, start=(j == 0), stop=(j == 3))
                    nc.scalar.copy(out_sp[cc][:, sl], psT[:, :])
                for oc in range(2):
                    py = ps.tile([128, 512], F32, tag="st", bufs=3, name=f"py{oc}_{g}")
                    nc.tensor.matmul(out=py[:, :], lhsT=wr("wp", 0, oc),
                                     rhs=out_sp[0][:, sl],
                                     start=True, stop=False)
                    nc.tensor.matmul(out=py[:, :], lhsT=wr("wp", 1, oc),
                                     rhs=out_sp[1][:, sl],
                                     start=False, stop=True)
                    y_sb = pout.tile([128, 512], F32, tag="y", name=f"y_sb{oc}_{g}")
                    nc.vector.tensor_scalar_add(y_sb[:, :], py[:, :],
                                                bias_sb[:, oc:oc + 1])
                    nc.sync.dma_start(out=y_d[oc * 128:(oc + 1) * 128, sl],
                                      in_=y_sb[:, :])
                ctx.pop_all().close() if False else None

    nc.compile()
    return nc


_CACHE = {}


def _get_program():
    if "nc" not in _CACHE:
        op = register_expq_op()
        _CACHE["nc"] = build_program(op)
    return _CACHE["nc"]


_IDENT = np.eye(128, dtype=np.float32)


def make_in_maps(x, w_qkv, w_proj, b_proj):
    x2 = x.reshape(B, C, N)
    wq_t = np.ascontiguousarray((w_qkv[0:C] / 128.0).T)
    wk_t = np.ascontiguousarray(w_qkv[C:2 * C].T)
    wv_t = np.ascontiguousarray(w_qkv[2 * C:3 * C].T)
    wp_t = np.ascontiguousarray(w_proj.T)
    bias2 = np.ascontiguousarray(b_proj.reshape(2, 128).T)
    in_maps = []
    for core in range(8):
        b, half = divmod(core, 2)
        n0 = half * NQ
        x_rot = np.concatenate([x2[b][:, n0:], x2[b][:, :n0]], axis=1)
        in_maps.append({
            "x": np.ascontiguousarray(x_rot),
            "wq": wq_t, "wk": wk_t, "wv": wv_t, "wp": wp_t,
            "bias": bias2, "ident": _IDENT,
        })
    return in_maps


def kernel(x, w_qkv, w_proj, b_proj):
    x = np.asarray(x, np.float32)
    w_qkv = np.asarray(w_qkv, np.float32)
    w_proj = np.asarray(w_proj, np.float32)
    b_proj = np.asarray(b_proj, np.float32)

    nc = _get_program()
    in_maps = make_in_maps(x, w_qkv, w_proj, b_proj)
    res = run_bass_kernel_spmd(nc, in_maps, list(range(8)))

    y = np.empty((B, C, N), np.float32)
    for core in range(8):
        b, half = divmod(core, 2)
        n0 = half * NQ
        y[b][:, n0:n0 + NQ] = res.results[core]["y"]
    return y.reshape(B, C, H, W)


# revision 38
# speedup vs baseline: 1.0175x; 1.0175x over previous
"""AttentionBlock (1x1-conv QKV + 4-head softmax attention + 1x1-conv proj)
on 8 Trainium2 NeuronCores.

Sharding: data-parallel over (batch b, query-half h) -> 8 shards. Each core
gets x rotated so its 2048 query columns are always columns 0:2048 (key order
is a permutation, which softmax-attention is invariant to), computes
qkv projections, 4-head attention for its half of the queries, and the output
projection for its [256, 2048] output slice. No collectives.

v2 structure (cost-model aware: matmul cost = streamed rhs columns):
  - scores S^T = K^T Q in f32r (1/16 pre-folded into w_q on the host so the
    DVE exp polynomial stays in range), 256-query tiles, keys-major PSUM.
    q/k live in per-head partition-0 tiles: mixed-partition-offset matmul
    operands crash the walrus/HW path.
  - exp split by column between Act (native Exp, scale=16) and a two-instr
    DVE pipeline (EXPQ2A: minimax-quartic^2 of exp(16t)*24, EXPQ2B: ^8),
    24^16 cancels in softmax since rowsums come from the same values. The
    per-nt column split keeps every softmax row on one implementation.
  - attn@V in O-form: out[query, dh] with rhs=[V_h | ones] so rowsums ride
    along as a 65th column; 65-col bf16 matmuls with 128-query-partition
    output (2x fewer streamed columns than the channel-major form). One
    PSUM accumulation group per 2KB bank (lazy zero-region semantics).
  - normalization per 128-query chunk on DVE (reciprocal + stride-0-broadcast
    tensor_tensor), then PE transposes O back to channel-major (identity
    rhs) for the output projection, pipelined per 512-query group.
  - f32r DRAM params + f32r SBUF tiles everywhere (no conversion copies);
    PSUM triple-buffered scores so the exp WAR chain stays off the critical
    path; EXPQ2B deprioritized so the next tile's EXPQ2A fills its ack gap.
"""
import os
import sys

sys.path.insert(0, '/opt/trn_rl_repo')

import numpy as np
from contextlib import ExitStack

from concourse import bass, bacc, mybir
import concourse.tile as tile
from concourse import dve_ops
from concourse.dve_ops import DveOp, OPS, CUSTOM_DVE_SPECS, _SUB_OPCODE_FOR_NAME
from concourse.dve_spec import Spec, Src0, C0, C1, C2, C3, lower, sq, _spill_c3_to_src1
from concourse.dve_uop import DveOpSpec
from concourse.bass_utils import run_bass_kernel_spmd

F32 = mybir.dt.float32
F32R = mybir.dt.float32r
BF16 = mybir.dt.bfloat16
ActFn = mybir.ActivationFunctionType

B, C, H, W = 4, 256, 64, 64
HEADS, DH = 4, 64
N = H * W            # 4096 keys
NQ = N // 2          # 2048 queries per core
NT = 256             # phase-2 query tile
N_NT = NQ // NT      # 8
N_MC = N // 128      # 32 key chunks
VSTR = HEADS * (DH + 1)  # 260: per-mc vT stride ([V_h | ones] x 4 heads)

# exp(16t) * 24^16 ~ [(t^2 + c0 t + c1)(t^2 + c2 t + c3)]^16 for t in
# [-0.625, 0.625] (score x = 16t in [-10, 10]); max rel err ~9e-4. The
# 24^16 factor cancels in softmax normalization. Split into two DVE
# instructions: EXPQ2A computes P^2 (quartic + one square, 8 ALU ops),
# EXPQ2B cubes the squaring three more times ((P^2)^8 = P^16).
EQ = (0.5504330780327099, 6.148042182109957,
      3.5525352677618507, 3.903596315668177)

# Act exp column count (0..1024) per nt slot; the remaining columns of every
# score tile go to the DVE quartic pipeline. Uniform split keeps both engines
# busy on every tile (whole-tile alternation serializes the engines in time).
EXP_ACOLS = [int(v) for v in os.environ.get(
    "EXP_ACOLS", "765,765,765,765,765,765,765,765").split(",")]
assert len(EXP_ACOLS) == 8


def _ref_expq2a(in0, in1, c0, c1, c2):
    x = in0.astype(np.float32)
    c3 = in1.astype(np.float32) if isinstance(in1, np.ndarray) else np.float32(in1)
    p = (((x + np.float32(c0)) * x + np.float32(c1))
         * ((x + np.float32(c2)) * x + c3)).astype(np.float32)
    return (p * p).astype(np.float32)


def _ref_expq2b(in0, in1, c0, c1, c2):
    p = in0.astype(np.float32)
    for _ in range(3):
        p = (p * p).astype(np.float32)
    return p


def _register(name, spec, rd1_en):
    row = dve_ops._CUSTOM_DVE_ROW_BASE + len(OPS)
    assert row < 0x20
    _SUB_OPCODE_FOR_NAME[name] = row
    shas = {}
    for ver in ("v3", "v4"):
        uops = lower(spec, ver=ver)
        shas[ver] = DveOpSpec(name=name, opcode=row, uops=uops, rd1_en=rd1_en).sha(ver)
    op = DveOp(name, spec, subdim=False, uops_sha=shas)
    OPS.append(op)
    CUSTOM_DVE_SPECS[name] = spec
    return op


def register_expq_op():
    if "EXPQ2A_ANT" in _SUB_OPCODE_FOR_NAME:
        a = next(op for op in OPS if op.name == "EXPQ2A_ANT")
        b = next(op for op in OPS if op.name == "EXPQ2B_ANT")
        return a, b
    x = Src0
    body_a = _spill_c3_to_src1(
        sq(((x + C0) * x + C1) * ((x + C2) * x + C3)))
    op_a = _register("EXPQ2A_ANT", Spec(body=body_a, reference=_ref_expq2a), True)
    body_b = sq(sq(sq(x)))
    op_b = _register("EXPQ2B_ANT", Spec(body=body_b, reference=_ref_expq2b), False)
    return op_a, op_b


def _ap3(base_ap, dims):
    """Manual AP with the partition dim of base_ap plus custom free dims."""
    return bass.AP(tensor=base_ap.tensor, offset=base_ap.offset,
                   ap=[list(base_ap.ap[0])] + [list(d) for d in dims])


def build_program(expq_op):
    nc = bacc.Bacc(target_bir_lowering=False)

    x_d = nc.declare_dram_parameter("x", [C, N], F32R, isOutput=False)
    wq_d = nc.declare_dram_parameter("wq", [C, C], F32R, isOutput=False)
    wk_d = nc.declare_dram_parameter("wk", [C, C], F32R, isOutput=False)
    wv_d = nc.declare_dram_parameter("wv", [C, C], F32R, isOutput=False)
    wp_d = nc.declare_dram_parameter("wp", [C, C], F32R, isOutput=False)
    bias_d = nc.declare_dram_parameter("bias", [128, 2], F32, isOutput=False)
    id_d = nc.declare_dram_parameter("ident", [128, 128], F32R, isOutput=False)
    y_d = nc.declare_dram_parameter("y", [C, NQ], F32, isOutput=True)

    with tile.TileContext(nc) as tc, ExitStack() as ctx:
        sb = ctx.enter_context(tc.tile_pool(name="sb", bufs=1))
        pex = ctx.enter_context(tc.tile_pool(name="pex", bufs=3))
        pout = ctx.enter_context(tc.tile_pool(name="pout", bufs=2))
        ps = ctx.enter_context(tc.tile_pool(name="ps", bufs=1, space="PSUM"))

        # ---------------- loads (weights first so QKV can start early) -----
        XC = 512  # x DMA chunk width so phase 1 can start early
        w_sb = {}
        w_drams = {"wq": wq_d, "wk": wk_d, "wv": wv_d, "wp": wp_d}
        for name in w_drams:
            w_sb[name] = [sb.tile([128, C], F32R, tag=f"{name}{kc}", name=f"{name}f{kc}")
                          for kc in range(2)]
        x_f = [sb.tile([128, N], F32R, tag=f"xf{i}", name=f"xf{i}") for i in range(2)]

        def w_dma(name):
            for kc in range(2):
                nc.sync.dma_start(out=w_sb[name][kc],
                                  in_=w_drams[name][kc * 128:(kc + 1) * 128, :])

        def x_dma(ch):
            for kc in range(2):
                nc.sync.dma_start(out=x_f[kc][:, ch * XC:(ch + 1) * XC],
                                  in_=x_d[kc * 128:(kc + 1) * 128, ch * XC:(ch + 1) * XC])

        w_dma("wq")
        x_dma(0)
        w_dma("wk")
        x_dma(1)
        w_dma("wv")
        w_dma("wp")
        for ch in range(2, N // XC):
            x_dma(ch)
        bias_sb = sb.tile([128, 2], F32, tag="bias")
        nc.sync.dma_start(out=bias_sb, in_=bias_d[:, :])
        id_sb = sb.tile([128, 128], F32R, tag="id")
        nc.sync.dma_start(out=id_sb, in_=id_d[:, :])

        c3_t = sb.tile([128, 1], F32, tag="c3")
        nc.vector.memset(c3_t, float(EQ[3]))

        def xr(kc, sl):
            return x_f[kc][:, sl]

        def wr(name, kc, oc):
            return w_sb[name][kc][:, oc * 128:(oc + 1) * 128]

        # ---------------- phase 1: qkv projections ----------------
        # per-head tiles, always at partition offset 0 (mixed-partition-offset
        # matmul operands crash the walrus/HW path)
        q_sb = [sb.tile([64, NQ], F32R, tag=f"q{h}", name=f"q_sb{h}") for h in range(4)]
        k_sb = [sb.tile([64, N], F32R, tag=f"k{h}", name=f"k_sb{h}") for h in range(4)]
        vT_sb = sb.tile([128, N_MC * VSTR], BF16, tag="vT")

        # ones columns of vT (col 64 + 65*h + 260*mc), written once on Pool
        ones_ap = _ap3(vT_sb[:, DH:DH + 1], [[VSTR, N_MC], [DH + 1, HEADS]])
        nc.gpsimd.memset(ones_ap, 1.0)

        evac_i = [0]

        def evac_copy(out_ap, in_ap):
            # alternate PSUM evacuations between Act and DVE
            eng = nc.scalar.copy if evac_i[0] % 2 == 0 else nc.vector.tensor_copy
            evac_i[0] += 1
            return eng(out_ap, in_ap)

        for oc in range(2):
            for t4 in range(4):
                pq = ps.tile([128, 512], F32, tag="st", bufs=3, name=f"pq{oc}_{t4}")
                sl = slice(t4 * 512, (t4 + 1) * 512)
                nc.tensor.matmul(out=pq[:, :], lhsT=wr("wq", 0, oc), rhs=xr(0, sl),
                                 start=True, stop=False)
                nc.tensor.matmul(out=pq[:, :], lhsT=wr("wq", 1, oc), rhs=xr(1, sl),
                                 start=False, stop=True)
                evac_copy(q_sb[2 * oc][:, sl], pq[0:64, :])
                evac_copy(q_sb[2 * oc + 1][:, sl], pq[64:128, :])
        for oc in range(2):
            for t8 in range(8):
                pk = ps.tile([128, 512], F32, tag="st", bufs=3, name=f"pk{oc}_{t8}")
                sl = slice(t8 * 512, (t8 + 1) * 512)
                nc.tensor.matmul(out=pk[:, :], lhsT=wr("wk", 0, oc), rhs=xr(0, sl),
                                 start=True, stop=False)
                nc.tensor.matmul(out=pk[:, :], lhsT=wr("wk", 1, oc), rhs=xr(1, sl),
                                 start=False, stop=True)
                evac_copy(k_sb[2 * oc][:, sl], pk[0:64, :])
                evac_copy(k_sb[2 * oc + 1][:, sl], pk[64:128, :])
        for mc in range(N_MC):
            pv = ps.tile([128, 256], F32, tag="st", bufs=3, name=f"pv{mc}")
            msl = slice(mc * 128, (mc + 1) * 128)
            nc.tensor.matmul(out=pv[:, :], lhsT=xr(0, msl), rhs=w_sb["wv"][0][:, :],
                             start=True, stop=False)
            nc.tensor.matmul(out=pv[:, :], lhsT=xr(1, msl), rhs=w_sb["wv"][1][:, :],
                             start=False, stop=True)
            # strided copy into the [V_h | ones] layout: col 65*h + d
            vout = _ap3(vT_sb[:, mc * VSTR:mc * VSTR + 1], [[DH + 1, HEADS], [1, DH]])
            vin = _ap3(pv[:, 0:1], [[DH, HEADS], [1, DH]])
            evac_copy(vout, vin)

        # ---------------- phase 2: attention ----------------
        o_n = sb.tile([128, 16 * 256], F32R, tag="on")   # normalized O, [q, c]
        out_sp = [sb.tile([128, NQ], F32R, tag=f"osp{oc}", name=f"osp{oc}") for oc in range(2)]

        op_a, op_b = expq_op
        for nt in range(N_NT):               # 256-query tiles
            qsl = slice(nt * NT, (nt + 1) * NT)
            O_ps = [ps.tile([128, 512], F32, tag="o", bufs=2, name=f"O{nt}_{qs}")
                    for qs in range(2)]
            for mc in range(N_MC):
                msl = slice(mc * 128, (mc + 1) * 128)
                # all 4 heads' scores for this (nt, mc) in one 2-bank tile;
                # triple-buffered so the exp WAR chain stays off the
                # critical path.
                pst = ps.tile([128, 1024], F32, tag="st", bufs=3,
                              name=f"pst{nt}_{mc}")
                for h in range(4):
                    # per-head operands at partition offset 0; two heads per
                    # 2KB PSUM bank: first starts the group (lazy-zeroing the
                    # bank), second stops it.
                    nc.tensor.matmul(out=pst[:, h * 256:(h + 1) * 256],
                                     lhsT=k_sb[h][:, msl],
                                     rhs=q_sb[h][:, qsl],
                                     start=(h % 2 == 0), stop=(h % 2 == 1))
                et = pex.tile([128, 1024], BF16, tag="et", name=f"et{nt}_{mc}")
                acols = EXP_ACOLS[nt]
                if acols > 0:
                    nc.scalar.activation(et[:, 0:acols], pst[:, 0:acols],
                                         ActFn.Exp, scale=16.0)
                if acols < 1024:
                    y1 = pex.tile([128, 1024], F32, tag="y1", name=f"y1{nt}_{mc}")
                    nc.vector._custom_dve(op_a, out=y1[:, acols:1024],
                                          in0=pst[:, acols:1024],
                                          in1=c3_t[:, :], s0=float(EQ[0]),
                                          s1=float(EQ[1]), imm2=float(EQ[2]))
                    # deprioritize the second stage so the scheduler slots the
                    # next tile's EXPQ2A into the A->B ack gap instead of
                    # idling the DVE on the y1 write-ack.
                    with tc.high_priority(-24):
                        nc.vector._custom_dve(op_b, out=et[:, acols:1024],
                                              in0=y1[:, acols:1024])
                first, last = mc == 0, mc == N_MC - 1
                for h in range(4):
                    for qs in range(2):
                        # one accumulation group per O bank: start only on the
                        # very first write (the zero-region covers all 4 heads'
                        # columns), stop only on the very last.
                        nc.tensor.matmul(
                            out=O_ps[qs][:, h * 128:h * 128 + DH + 1],
                            lhsT=et[:, h * 256 + qs * 128:h * 256 + qs * 128 + 128],
                            rhs=vT_sb[:, mc * VSTR + h * (DH + 1):mc * VSTR + (h + 1) * (DH + 1)],
                            start=(first and h == 0), stop=(last and h == 3))
            for qs in range(2):
                rcp = sb.tile([128, 4], F32, tag="rcp", bufs=2, name=f"rcp{nt}_{qs}")
                rs_ap = _ap3(O_ps[qs][:, DH:DH + 1], [[128, 4], [1, 1]])
                nc.vector.reciprocal_approx_fast(out=rcp[:, :], in_=rs_ap)
                qc = nt * 2 + qs
                o_out = _ap3(o_n[:, qc * 256:qc * 256 + 1], [[64, 4], [1, 64]])
                o_in = _ap3(O_ps[qs][:, 0:1], [[128, 4], [1, 64]])
                r_in = _ap3(rcp[:, 0:1], [[1, 4], [0, 64]])
                nc.vector.tensor_tensor(out=o_out, in0=o_in, in1=r_in,
                                        op=mybir.AluOpType.mult)
            if nt % 2 == 1:
                # transpose the last 4 qchunks back to channel-major and
                # project, pipelined with the next nt's attention. Deprioritized
                # so the next nt's S^T matmuls win the PE when both are ready.
                ctx.enter_context(tc.high_priority(-64))
                g = nt // 2
                sl = slice(g * 512, (g + 1) * 512)
                for cc in range(2):
                    psT = ps.tile([128, 512], F32R, tag="o", bufs=2,
                                  name=f"psT{g}_{cc}")
                    for j in range(4):
                        qc = g * 4 + j
                        nc.tensor.matmul(
                            out=psT[:, j * 128:(j + 1) * 128],
                            lhsT=o_n[:, qc * 256 + cc * 128:qc * 256 + cc * 128 + 128],
                            rhs=id_sb[:, :],
                            is_transpose=True, start=(j == 0), stop=(j == 3))
                    nc.scalar.copy(out_sp[cc][:, sl], psT[:, :])
                for oc in range(2):
                    py = ps.tile([128, 512], F32, tag="o", bufs=2, name=f"py{oc}_{g}")
                    nc.tensor.matmul(out=py[:, :], lhsT=wr("wp", 0, oc),
                                     rhs=out_sp[0][:, sl],
                                     start=True, stop=False)
                    nc.tensor.matmul(out=py[:, :], lhsT=wr("wp", 1, oc),
                                     rhs=out_sp[1][:, sl],
                                     start=False, stop=True)
                    y_sb = pout.tile([128, 512], F32, tag="y", name=f"y_sb{oc}_{g}")
                    nc.vector.tensor_scalar_add(y_sb[:, :], py[:, :],
                                                bias_sb[:, oc:oc + 1])
                    nc.sync.dma_start(out=y_d[oc * 128:(oc + 1) * 128, sl],
                                      in_=y_sb[:, :])
                ctx.pop_all().close() if False else None

    nc.compile()
    return nc


_CACHE = {}


def _get_program():
    if "nc" not in _CACHE:
        op = register_expq_op()
        _CACHE["nc"] = build_program(op)
    return _CACHE["nc"]


_IDENT = np.eye(128, dtype=np.float32)


def make_in_maps(x, w_qkv, w_proj, b_proj):
    x2 = x.reshape(B, C, N)
    wq_t = np.ascontiguousarray((w_qkv[0:C] / 128.0).T)
    wk_t = np.ascontiguousarray(w_qkv[C:2 * C].T)
    wv_t = np.ascontiguousarray(w_qkv[2 * C:3 * C].T)
    wp_t = np.ascontiguousarray(w_proj.T)
    bias2 = np.ascontiguousarray(b_proj.reshape(2, 128).T)
    in_maps = []
    for core in range(8):
        b, half = divmod(core, 2)
        n0 = half * NQ
        x_rot = np.concatenate([x2[b][:, n0:], x2[b][:, :n0]], axis=1)
        in_maps.append({
            "x": np.ascontiguousarray(x_rot),
            "wq": wq_t, "wk": wk_t, "wv": wv_t, "wp": wp_t,
            "bias": bias2, "ident": _IDENT,
        })
    return in_maps


def kernel(x, w_qkv, w_proj, b_proj):
    x = np.asarray(x, np.float32)
    w_qkv = np.asarray(w_qkv, np.float32)
    w_proj = np.asarray(w_proj, np.float32)
    b_proj = np.asarray(b_proj, np.float32)

    nc = _get_program()
    in_maps = make_in_maps(x, w_qkv, w_proj, b_proj)
    res = run_bass_kernel_spmd(nc, in_maps, list(range(8)))

    y = np.empty((B, C, N), np.float32)
    for core in range(8):
        b, half = divmod(core, 2)
        n0 = half * NQ
        y[b][:, n0:n0 + NQ] = res.results[core]["y"]
    return y.reshape(B, C, H, W)
